# revision 13
# baseline (speedup 1.0000x reference)
"""Trainium2 kernel for nn_Distiller column scatter.

Computes, for student and teacher logits (B, C) and index vector
seen_classes (C), the pair of (B, T) tensors with
out[:, seen_classes] = logits and zeros elsewhere.

Strategy (B=8192, C=5000, T=20000, 8 cores, batch-parallel):
  - Host: sort seen_classes; column-gather + transpose + block each
    core's row shard into lhsT tiles (sorted classes on partitions,
    rows on the free axis).  Build one 0/1 scatter matrix P (128, T)
    with P[k % 128, tgt[k]] = 1 for sorted index k.
  - Device: for each 128-row tile and each 128-column block of sorted
    classes, one PE transpose-mode matmul per <=512-wide output span
    chunk computes out_chunk = lhsT.T @ P[:, chunk] exactly (0/1
    moving operand -> bit-exact fp32 pass-through).  PSUM -> SBUF via
    alternating Vector/Scalar copies, then HWDGE DMA to DRAM.
  - Spans of consecutive sorted-class blocks tile [0, T) exactly, so
    every output element (zeros included) is written exactly once.
"""

import os
import sys

for _p in ("/root/.axon_site/_ro/trn_rl_repo", "/opt/trn_rl_repo"):
    if os.path.isdir(_p) and _p not in sys.path:
        sys.path.insert(0, _p)  # later inserts win: /opt preferred

import numpy as np

N_CORES = 8
B = 8192
C = 5000
T = 20000
ROWS_PER_CORE = B // N_CORES  # 1024
RT = 128  # rows per tile
NT = ROWS_PER_CORE // RT  # 8 row tiles per core
NB = (C + 127) // 128  # 40 sorted-class blocks
CPAD = NB * 128  # 5120
MAX_N = 512  # max moving free dim (fp32)
SLAB = 5000  # output staging slab width (T % SLAB == 0)
NSLAB = T // SLAB


def _build_plan(seen_classes):
    """Sort classes, derive per-block output spans and chunk splits."""
    seen = np.asarray(seen_classes).astype(np.int64).ravel()
    assert seen.shape == (C,)
    order = np.argsort(seen, kind="stable")
    tgt = seen[order]  # strictly increasing (unique ids)

    # span of block b: (end[b-1]+1 .. end[b]), first starts at 0,
    # last ends at T-1 -> spans tile [0, T) exactly.
    ends = np.empty(NB, dtype=np.int64)
    for b in range(NB):
        hi = min(128 * (b + 1), C)
        ends[b] = tgt[hi - 1]
    ends[NB - 1] = T - 1
    starts = np.empty(NB, dtype=np.int64)
    starts[0] = 0
    starts[1:] = ends[:-1] + 1

    # P matrix: one-hot columns by sorted index mod 128 (u8, cast-DMAed)
    pmat = np.zeros((128, T), dtype=np.uint8)
    pmat[np.arange(C) % 128, tgt] = 1

    # chunk splits (start, width) per block, each width <= MAX_N and
    # never crossing a SLAB-column boundary (output staging granularity)
    chunks = []
    for b in range(NB):
        end = int(ends[b])
        c0 = int(starts[b])
        bl = []
        while c0 <= end:
            nxt_slab = (c0 // SLAB + 1) * SLAB
            cw = min(MAX_N, end - c0 + 1, nxt_slab - c0)
            bl.append((c0, cw))
            c0 += cw
        chunks.append(bl)
    return order, pmat, chunks


def _block_shard(x, order, core):
    """(B, C) full input -> (NT, 128, NB, 128) [t, p, b, j] blocked lhsT
    layout for one core: value [t, p, b, j] = x[1024*core + 128*t + j,
    order[128*b + p]] with zero padding for 128*b + p >= C."""
    rows = x[ROWS_PER_CORE * core : ROWS_PER_CORE * (core + 1)]
    g = rows[:, order]  # (1024, C) sorted-column gather
    if CPAD != C:
        g = np.concatenate(
            [g, np.zeros((ROWS_PER_CORE, CPAD - C), dtype=np.float32)], axis=1
        )
    # (1024, CPAD) -> [t, j, b, p] -> [t, p, b, j]
    v = g.reshape(NT, RT, NB, 128).transpose(0, 3, 2, 1)
    return np.ascontiguousarray(v)


def _build_nc(chunks):
    import concourse.bacc as bacc
    import concourse.tile as tile
    from concourse import mybir

    nc = bacc.Bacc(
        "TRN2", target_bir_lowering=False, debug=False, num_devices=N_CORES
    )
    f32 = mybir.dt.float32

    xs_in = nc.dram_tensor("xs", [NT, 128, NB * 128], f32, kind="ExternalInput").ap()
    xt_in = nc.dram_tensor("xt", [NT, 128, NB * 128], f32, kind="ExternalInput").ap()
    p_in = nc.dram_tensor("p", [128, T], mybir.dt.uint8, kind="ExternalInput").ap()
    os_out = nc.dram_tensor(
        "os", [ROWS_PER_CORE, T], f32, kind="ExternalOutput"
    ).ap()
    ot_out = nc.dram_tensor(
        "ot", [ROWS_PER_CORE, T], f32, kind="ExternalOutput"
    ).ap()

    # flat chunk list in column order, annotated with owning block
    flat = []
    for b in range(NB):
        for c0, cw in chunks[b]:
            flat.append((b, c0, cw))
    flat.sort(key=lambda r: r[1])

    with tile.TileContext(nc) as tc:
        with (
            tc.tile_pool(name="pp", bufs=1) as pp,
            tc.tile_pool(name="xp", bufs=2) as xp,
            tc.tile_pool(name="sl", bufs=3) as sl,
            tc.tile_pool(name="ps", bufs=8, space="PSUM") as ps,
        ):
            p_tile = pp.tile([128, T], f32)
            nc.gpsimd.dma_start(p_tile[:], p_in[:])

            flip = 0
            for x_in, o_out in ((xs_in, os_out), (xt_in, ot_out)):
                for t in range(NT):
                    xtile = xp.tile([128, NB * 128], f32, tag="xtile")
                    nc.gpsimd.dma_start(xtile[:], x_in[t])
                    for s in range(NSLAB):
                        slab = sl.tile([128, SLAB], f32, tag="slab")
                        lo, hi = SLAB * s, SLAB * (s + 1)
                        for b, c0, cw in flat:
                            if c0 < lo or c0 >= hi:
                                continue
                            lhsT = xtile[:, 128 * b : 128 * (b + 1)]
                            acc = ps.tile([128, cw], f32, tag="acc")
                            nc.tensor.matmul(
                                acc[:],
                                lhsT,
                                p_tile[:, c0 : c0 + cw],
                                start=True,
                                stop=True,
                                is_transpose=True,
                            )
                            if flip == 0:
                                nc.vector.tensor_copy(
                                    slab[:, c0 - lo : c0 - lo + cw], acc[:]
                                )
                            else:
                                nc.scalar.copy(
                                    slab[:, c0 - lo : c0 - lo + cw], acc[:]
                                )
                            flip ^= 1
                        dma_eng = nc.sync if (t + s) % 2 == 0 else nc.scalar
                        dma_eng.dma_start(
                            o_out[128 * t : 128 * (t + 1), lo:hi], slab[:]
                        )
    nc.compile()
    return nc


def kernel(logits_student, logits_teacher, seen_classes, total_class):
    from concourse.bass_utils import run_bass_kernel_spmd

    xs = np.asarray(logits_student, dtype=np.float32)
    xt = np.asarray(logits_teacher, dtype=np.float32)
    assert xs.shape == (B, C) and xt.shape == (B, C)
    assert int(total_class) == T

    order, pmat, chunks = _build_plan(seen_classes)
    nc = _build_nc(chunks)

    in_maps = []
    for core in range(N_CORES):
        in_maps.append(
            {
                "xs": _block_shard(xs, order, core).reshape(NT, 128, NB * 128),
                "xt": _block_shard(xt, order, core).reshape(NT, 128, NB * 128),
                "p": pmat,
            }
        )

    kernel.last_nc = nc  # for test harness introspection (TimelineSim)
    res = run_bass_kernel_spmd(nc, in_maps, core_ids=list(range(N_CORES)))
    kernel.last_results = res

    new_s = np.concatenate([res.results[i]["os"] for i in range(N_CORES)], axis=0)
    new_t = np.concatenate([res.results[i]["ot"] for i in range(N_CORES)], axis=0)
    return (new_s, new_t)


# revision 24
# speedup vs baseline: 1.0118x; 1.0118x over previous
"""Trainium2 kernel for nn_Distiller column scatter.

Computes, for student and teacher logits (B, C) and index vector
seen_classes (C), the pair of (B, T) tensors with
out[:, seen_classes] = logits and zeros elsewhere.

Strategy (B=8192, C=5000, T=20000, 8 cores, batch-parallel):
  - Host: sort seen_classes; column-gather + transpose + block each
    core's row shard into lhsT tiles (sorted classes on partitions,
    rows on the free axis).  Build one 0/1 scatter matrix P (128, T)
    with P[k % 128, tgt[k]] = 1 for sorted index k.
  - Device: for each 128-row tile and each 128-column block of sorted
    classes, one PE transpose-mode matmul per <=512-wide output span
    chunk computes out_chunk = lhsT.T @ P[:, chunk] exactly (0/1
    moving operand -> bit-exact fp32 pass-through).  PSUM -> SBUF via
    alternating Vector/Scalar copies, then HWDGE DMA to DRAM.
  - Spans of consecutive sorted-class blocks tile [0, T) exactly, so
    every output element (zeros included) is written exactly once.
"""

import os
import sys

for _p in ("/root/.axon_site/_ro/trn_rl_repo", "/opt/trn_rl_repo"):
    if os.path.isdir(_p) and _p not in sys.path:
        sys.path.insert(0, _p)  # later inserts win: /opt preferred

import numpy as np

N_CORES = 8
B = 8192
C = 5000
T = 20000
ROWS_PER_CORE = B // N_CORES  # 1024
RT = 128  # rows per tile
NT = ROWS_PER_CORE // RT  # 8 row tiles per core
NB = (C + 127) // 128  # 40 sorted-class blocks
CPAD = NB * 128  # 5120
MAX_N = 512  # max moving free dim (fp32)
SLAB = 5000  # output staging slab width (T % SLAB == 0)
NSLAB = T // SLAB


def _build_plan(seen_classes):
    """Sort classes, derive per-block output spans and chunk splits."""
    seen = np.asarray(seen_classes).astype(np.int64).ravel()
    assert seen.shape == (C,)
    order = np.argsort(seen, kind="stable")
    tgt = seen[order]  # strictly increasing (unique ids)

    # span of block b: (end[b-1]+1 .. end[b]), first starts at 0,
    # last ends at T-1 -> spans tile [0, T) exactly.
    ends = np.empty(NB, dtype=np.int64)
    for b in range(NB):
        hi = min(128 * (b + 1), C)
        ends[b] = tgt[hi - 1]
    ends[NB - 1] = T - 1
    starts = np.empty(NB, dtype=np.int64)
    starts[0] = 0
    starts[1:] = ends[:-1] + 1

    # per-column sorted-index-mod-128 (or -1 for non-target columns);
    # P is built on device as (iota_p == pidx_c)
    pidx = np.full((1, T), -1.0, dtype=np.float32)
    pidx[0, tgt] = (np.arange(C) % 128).astype(np.float32)

    # chunk splits (start, width) per block, each width <= MAX_N and
    # never crossing a SLAB-column boundary (output staging granularity)
    chunks = []
    for b in range(NB):
        end = int(ends[b])
        c0 = int(starts[b])
        bl = []
        while c0 <= end:
            nxt_slab = (c0 // SLAB + 1) * SLAB
            cw = min(MAX_N, end - c0 + 1, nxt_slab - c0)
            bl.append((c0, cw))
            c0 += cw
        chunks.append(bl)
    return order, pidx, chunks


def _block_shard(x, order, core):
    """(B, C) full input -> (NT, 128, NB, 128) [t, p, b, j] blocked lhsT
    layout for one core: value [t, p, b, j] = x[1024*core + 128*t + j,
    order[128*b + p]] with zero padding for 128*b + p >= C."""
    rows = x[ROWS_PER_CORE * core : ROWS_PER_CORE * (core + 1)]
    g = rows[:, order]  # (1024, C) sorted-column gather
    if CPAD != C:
        g = np.concatenate(
            [g, np.zeros((ROWS_PER_CORE, CPAD - C), dtype=np.float32)], axis=1
        )
    # (1024, CPAD) -> [t, j, b, p] -> [t, p, b, j]
    v = g.reshape(NT, RT, NB, 128).transpose(0, 3, 2, 1)
    return np.ascontiguousarray(v)


def _build_nc(chunks):
    import concourse.bacc as bacc
    import concourse.tile as tile
    from concourse import mybir

    nc = bacc.Bacc(
        "TRN2", target_bir_lowering=False, debug=False, num_devices=N_CORES
    )
    f32 = mybir.dt.float32

    xs_in = nc.dram_tensor("xs", [NT, 128, NB * 128], f32, kind="ExternalInput").ap()
    xt_in = nc.dram_tensor("xt", [NT, 128, NB * 128], f32, kind="ExternalInput").ap()
    pidx_in = nc.dram_tensor("pidx", [1, T], f32, kind="ExternalInput").ap()
    iota_in = nc.dram_tensor("iota", [128, 1], f32, kind="ExternalInput").ap()
    os_out = nc.dram_tensor(
        "os", [ROWS_PER_CORE, T], f32, kind="ExternalOutput"
    ).ap()
    ot_out = nc.dram_tensor(
        "ot", [ROWS_PER_CORE, T], f32, kind="ExternalOutput"
    ).ap()

    # flat chunk list in column order, annotated with owning block
    flat = []
    for b in range(NB):
        for c0, cw in chunks[b]:
            flat.append((b, c0, cw))
    flat.sort(key=lambda r: r[1])

    with tile.TileContext(nc) as tc:
        with (
            tc.tile_pool(name="pp", bufs=1) as pp,
            tc.tile_pool(name="xp", bufs=4) as xp,
            tc.tile_pool(name="sl", bufs=3) as sl,
            tc.tile_pool(name="ps", bufs=8, space="PSUM") as ps,
        ):
            # build the scatter matrix P on device: P[p, c] = (pidx[c] == p).
            # pidx rows are streamed in small chunks, partition-broadcast
            # into the P quarter, then compared in place against the iota
            # column.  P lives as one tile per SLAB quarter so main-loop
            # matmuls only depend on their own quarter's build.
            p_q = [
                pp.tile([128, SLAB], f32, name=f"pq{q}") for q in range(NSLAB)
            ]
            iota_t = pp.tile([128, 1], f32, name="iota_t")
            nc.sync.dma_start(iota_t[:], iota_in[:])
            from concourse import mybir as _mb

            PBW = 1250
            for q in range(T // PBW):
                lo, hi = PBW * q, PBW * (q + 1)
                pt = p_q[lo // SLAB]
                plo = lo - (lo // SLAB) * SLAB
                pidx_c = pp.tile([1, PBW], f32, tag="pidx_c", bufs=2, name=f"px{q}")
                nc.sync.dma_start(pidx_c[:], pidx_in[0:1, lo:hi])
                nc.gpsimd.partition_broadcast(pt[:, plo : plo + PBW], pidx_c[:])
                nc.vector.tensor_scalar(
                    pt[:, plo : plo + PBW],
                    pt[:, plo : plo + PBW],
                    iota_t[:, 0:1],
                    None,
                    op0=_mb.AluOpType.is_equal,
                )

            HB = NB // 2  # blocks per half-load
            flip = 0
            for x_in, o_out in ((xs_in, os_out), (xt_in, ot_out)):
                for t in range(NT):
                    xtileA = xp.tile([128, HB * 128], f32, tag="xtile")
                    xtileB = xp.tile([128, (NB - HB) * 128], f32, tag="xtile")
                    nc.gpsimd.dma_start(xtileA[:], x_in[t, :, 0 : HB * 128])
                    nc.gpsimd.dma_start(xtileB[:], x_in[t, :, HB * 128 :])
                    for s in range(NSLAB):
                        slab = sl.tile([128, SLAB], f32, tag="slab")
                        lo, hi = SLAB * s, SLAB * (s + 1)
                        for b, c0, cw in flat:
                            if c0 < lo or c0 >= hi:
                                continue
                            if b < HB:
                                lhsT = xtileA[:, 128 * b : 128 * (b + 1)]
                            else:
                                lhsT = xtileB[
                                    :, 128 * (b - HB) : 128 * (b - HB + 1)
                                ]
                            acc = ps.tile([128, cw], f32, tag="acc")
                            nc.tensor.matmul(
                                acc[:],
                                lhsT,
                                p_q[s][:, c0 - lo : c0 - lo + cw],
                                start=True,
                                stop=True,
                                is_transpose=True,
                            )
                            if flip == 0:
                                nc.vector.tensor_copy(
                                    slab[:, c0 - lo : c0 - lo + cw], acc[:]
                                )
                            else:
                                nc.scalar.copy(
                                    slab[:, c0 - lo : c0 - lo + cw], acc[:]
                                )
                            flip ^= 1
                        dma_eng = nc.sync if (t + s) % 2 == 0 else nc.scalar
                        dma_eng.dma_start(
                            o_out[128 * t : 128 * (t + 1), lo:hi], slab[:]
                        )
    nc.compile()
    return nc


def kernel(logits_student, logits_teacher, seen_classes, total_class):
    from concourse.bass_utils import run_bass_kernel_spmd

    xs = np.asarray(logits_student, dtype=np.float32)
    xt = np.asarray(logits_teacher, dtype=np.float32)
    assert xs.shape == (B, C) and xt.shape == (B, C)
    assert int(total_class) == T

    order, pidx, chunks = _build_plan(seen_classes)
    nc = _build_nc(chunks)

    iota = np.arange(128, dtype=np.float32).reshape(128, 1)
    in_maps = []
    for core in range(N_CORES):
        in_maps.append(
            {
                "xs": _block_shard(xs, order, core).reshape(NT, 128, NB * 128),
                "xt": _block_shard(xt, order, core).reshape(NT, 128, NB * 128),
                "pidx": pidx,
                "iota": iota,
            }
        )

    kernel.last_nc = nc  # for test harness introspection (TimelineSim)
    res = run_bass_kernel_spmd(nc, in_maps, core_ids=list(range(N_CORES)))
    kernel.last_results = res

    new_s = np.concatenate([res.results[i]["os"] for i in range(N_CORES)], axis=0)
    new_t = np.concatenate([res.results[i]["ot"] for i in range(N_CORES)], axis=0)
    return (new_s, new_t)


# revision 28
# speedup vs baseline: 1.0485x; 1.0362x over previous
"""Trainium2 kernel for nn_Distiller column scatter.

Computes, for student and teacher logits (B, C) and index vector
seen_classes (C), the pair of (B, T) tensors with
out[:, seen_classes] = logits and zeros elsewhere.

Strategy (B=8192, C=5000, T=20000, 8 cores, batch-parallel):
  - Host: sort seen_classes; column-gather + transpose + block each
    core's row shard into lhsT tiles (sorted classes on partitions,
    rows on the free axis).  Build one 0/1 scatter matrix P (128, T)
    with P[k % 128, tgt[k]] = 1 for sorted index k.
  - Device: for each 128-row tile and each 128-column block of sorted
    classes, one PE transpose-mode matmul per <=512-wide output span
    chunk computes out_chunk = lhsT.T @ P[:, chunk] exactly (0/1
    moving operand -> bit-exact fp32 pass-through).  PSUM -> SBUF via
    alternating Vector/Scalar copies, then HWDGE DMA to DRAM.
  - Spans of consecutive sorted-class blocks tile [0, T) exactly, so
    every output element (zeros included) is written exactly once.
"""

import os
import sys

for _p in ("/root/.axon_site/_ro/trn_rl_repo", "/opt/trn_rl_repo"):
    if os.path.isdir(_p) and _p not in sys.path:
        sys.path.insert(0, _p)  # later inserts win: /opt preferred

import numpy as np

N_CORES = 8
B = 8192
C = 5000
T = 20000
ROWS_PER_CORE = B // N_CORES  # 1024
RT = 128  # rows per tile
NT = ROWS_PER_CORE // RT  # 8 row tiles per core
NB = (C + 127) // 128  # 40 sorted-class blocks
CPAD = NB * 128  # 5120
MAX_N = 512  # max moving free dim (fp32)
SLAB = 2500  # output staging slab width (T % SLAB == 0)
NSLAB = T // SLAB


def _build_plan(seen_classes):
    """Sort classes, derive per-block output spans and chunk splits."""
    seen = np.asarray(seen_classes).astype(np.int64).ravel()
    assert seen.shape == (C,)
    order = np.argsort(seen, kind="stable")
    tgt = seen[order]  # strictly increasing (unique ids)

    # span of block b: (end[b-1]+1 .. end[b]), first starts at 0,
    # last ends at T-1 -> spans tile [0, T) exactly.
    ends = np.empty(NB, dtype=np.int64)
    for b in range(NB):
        hi = min(128 * (b + 1), C)
        ends[b] = tgt[hi - 1]
    ends[NB - 1] = T - 1
    starts = np.empty(NB, dtype=np.int64)
    starts[0] = 0
    starts[1:] = ends[:-1] + 1

    # per-column sorted-index-mod-128 (or -1 for non-target columns);
    # P is built on device as (iota_p == pidx_c)
    pidx = np.full((1, T), -1.0, dtype=np.float32)
    pidx[0, tgt] = (np.arange(C) % 128).astype(np.float32)

    # chunk splits (start, width) per block, each width <= MAX_N and
    # never crossing a SLAB-column boundary (output staging granularity)
    chunks = []
    for b in range(NB):
        end = int(ends[b])
        c0 = int(starts[b])
        bl = []
        while c0 <= end:
            nxt_slab = (c0 // SLAB + 1) * SLAB
            cw = min(MAX_N, end - c0 + 1, nxt_slab - c0)
            bl.append((c0, cw))
            c0 += cw
        chunks.append(bl)
    return order, pidx, chunks


def _block_shard(x, order, core):
    """(B, C) full input -> (NT, 128, NB, 128) [t, p, b, j] blocked lhsT
    layout for one core: value [t, p, b, j] = x[1024*core + 128*t + j,
    order[128*b + p]] with zero padding for 128*b + p >= C."""
    rows = x[ROWS_PER_CORE * core : ROWS_PER_CORE * (core + 1)]
    g = rows[:, order]  # (1024, C) sorted-column gather
    if CPAD != C:
        g = np.concatenate(
            [g, np.zeros((ROWS_PER_CORE, CPAD - C), dtype=np.float32)], axis=1
        )
    # (1024, CPAD) -> [t, j, b, p] -> [t, p, b, j]
    v = g.reshape(NT, RT, NB, 128).transpose(0, 3, 2, 1)
    return np.ascontiguousarray(v)


def _build_nc(chunks):
    import concourse.bacc as bacc
    import concourse.tile as tile
    from concourse import mybir

    nc = bacc.Bacc(
        "TRN2", target_bir_lowering=False, debug=False, num_devices=N_CORES
    )
    f32 = mybir.dt.float32

    xs_in = nc.dram_tensor("xs", [NT, 128, NB * 128], f32, kind="ExternalInput").ap()
    xt_in = nc.dram_tensor("xt", [NT, 128, NB * 128], f32, kind="ExternalInput").ap()
    pidx_in = nc.dram_tensor("pidx", [1, T], f32, kind="ExternalInput").ap()
    iota_in = nc.dram_tensor("iota", [128, 1], f32, kind="ExternalInput").ap()
    os_out = nc.dram_tensor(
        "os", [ROWS_PER_CORE, T], f32, kind="ExternalOutput"
    ).ap()
    ot_out = nc.dram_tensor(
        "ot", [ROWS_PER_CORE, T], f32, kind="ExternalOutput"
    ).ap()

    # flat chunk list in column order, annotated with owning block
    flat = []
    for b in range(NB):
        for c0, cw in chunks[b]:
            flat.append((b, c0, cw))
    flat.sort(key=lambda r: r[1])

    with tile.TileContext(nc) as tc:
        with (
            tc.tile_pool(name="pp", bufs=1) as pp,
            tc.tile_pool(name="xp", bufs=12) as xp,
            tc.tile_pool(name="sl", bufs=4) as sl,
            tc.tile_pool(name="ps", bufs=8, space="PSUM") as ps,
        ):
            # build the scatter matrix P on device: P[p, c] = (pidx[c] == p).
            # pidx rows are streamed in small chunks, partition-broadcast
            # into the P quarter, then compared in place against the iota
            # column.  P lives as one tile per SLAB quarter so main-loop
            # matmuls only depend on their own quarter's build.
            p_q = [
                pp.tile([128, SLAB], f32, name=f"pq{q}") for q in range(NSLAB)
            ]
            iota_t = pp.tile([128, 1], f32, name="iota_t")
            nc.sync.dma_start(iota_t[:], iota_in[:])
            from concourse import mybir as _mb

            PBW = 1250
            for q in range(T // PBW):
                lo, hi = PBW * q, PBW * (q + 1)
                pt = p_q[lo // SLAB]
                plo = lo - (lo // SLAB) * SLAB
                pidx_c = pp.tile([1, PBW], f32, tag="pidx_c", bufs=2, name=f"px{q}")
                nc.sync.dma_start(pidx_c[:], pidx_in[0:1, lo:hi])
                nc.gpsimd.partition_broadcast(pt[:, plo : plo + PBW], pidx_c[:])
                nc.vector.tensor_scalar(
                    pt[:, plo : plo + PBW],
                    pt[:, plo : plo + PBW],
                    iota_t[:, 0:1],
                    None,
                    op0=_mb.AluOpType.is_equal,
                )

            HB = NB // 4  # blocks per quarter-load
            flip = 0
            for x_in, o_out in ((xs_in, os_out), (xt_in, ot_out)):
                for t in range(NT):
                    xparts = []
                    for h in range(4):
                        xq = xp.tile([128, HB * 128], f32, tag="xtile")
                        nc.gpsimd.dma_start(
                            xq[:], x_in[t, :, HB * 128 * h : HB * 128 * (h + 1)]
                        )
                        xparts.append(xq)
                    for s in range(NSLAB):
                        slab = sl.tile([128, SLAB], f32, tag="slab")
                        lo, hi = SLAB * s, SLAB * (s + 1)
                        for b, c0, cw in flat:
                            if c0 < lo or c0 >= hi:
                                continue
                            lhsT = xparts[b // HB][
                                :, 128 * (b % HB) : 128 * (b % HB + 1)
                            ]
                            acc = ps.tile([128, cw], f32, tag="acc")
                            nc.tensor.matmul(
                                acc[:],
                                lhsT,
                                p_q[s][:, c0 - lo : c0 - lo + cw],
                                start=True,
                                stop=True,
                                is_transpose=True,
                            )
                            if flip == 0:
                                nc.vector.tensor_copy(
                                    slab[:, c0 - lo : c0 - lo + cw], acc[:]
                                )
                            else:
                                nc.scalar.copy(
                                    slab[:, c0 - lo : c0 - lo + cw], acc[:]
                                )
                            flip ^= 1
                        dma_eng = nc.sync if (t + s) % 2 == 0 else nc.scalar
                        dma_eng.dma_start(
                            o_out[128 * t : 128 * (t + 1), lo:hi], slab[:]
                        )
    nc.compile()
    return nc


def kernel(logits_student, logits_teacher, seen_classes, total_class):
    from concourse.bass_utils import run_bass_kernel_spmd

    xs = np.asarray(logits_student, dtype=np.float32)
    xt = np.asarray(logits_teacher, dtype=np.float32)
    assert xs.shape == (B, C) and xt.shape == (B, C)
    assert int(total_class) == T

    order, pidx, chunks = _build_plan(seen_classes)
    nc = _build_nc(chunks)

    iota = np.arange(128, dtype=np.float32).reshape(128, 1)
    in_maps = []
    for core in range(N_CORES):
        in_maps.append(
            {
                "xs": _block_shard(xs, order, core).reshape(NT, 128, NB * 128),
                "xt": _block_shard(xt, order, core).reshape(NT, 128, NB * 128),
                "pidx": pidx,
                "iota": iota,
            }
        )

    kernel.last_nc = nc  # for test harness introspection (TimelineSim)
    res = run_bass_kernel_spmd(nc, in_maps, core_ids=list(range(N_CORES)))
    kernel.last_results = res

    new_s = np.concatenate([res.results[i]["os"] for i in range(N_CORES)], axis=0)
    new_t = np.concatenate([res.results[i]["ot"] for i in range(N_CORES)], axis=0)
    return (new_s, new_t)
